# revision 3
# baseline (speedup 1.0000x reference)
"""CrossScaleAttention (GNN segment-softmax attention) on 8 TRN2 NeuronCores.

Math refactor (all FLOPs on device, host only re-lays-out raw inputs):
  score[e] = Q[dst_e] . K[src_e],  Q = dst @ Wq^T + bq,  K = src @ Wk^T + bk
           = Q'[dst_e] . src_feat[src_e] + Q[dst_e].bk   (Q' = Q @ Wk)
  The per-dst constant Q[d].bk cancels in the segment softmax, so only
  Q' (a small per-dst table, computed on device) and raw src_feat rows are
  needed per edge.  Likewise the V projection commutes with the attention-
  weighted sum: out[d] = (sum_e attn_e * src_feat[src_e]) @ Wv^T + bv.

Sharding: dst nodes range-partitioned across 8 cores; edges sorted by dst on
host and laid out edge-major (128-edge tiles x 128-dst blocks, padded to a
uniform tile count per block).  The host ships src_feat rows in both [e, fi]
and [fi, e] tile orientations so the device streams them contiguously at full
DMA bandwidth (no indirect DMA).  Per tile the device computes all-pairs
scores (one matmul), masks+exps them (DVE+ACT), and accumulates the weighted
segment sums (one matmul into PSUM).
"""
import sys
sys.path.insert(0, "/opt/trn_rl_repo")

import numpy as np

import concourse.bass as bass
import concourse.bacc as bacc
import concourse.tile as tile
import concourse.mybir as mybir

N_NODES = 50000
D = 128
N_CORES = 8
NDST_CORE = N_NODES // N_CORES          # 6250
DBLK = 128
NBLK = (NDST_CORE + DBLK - 1) // DBLK   # 49
NDST_PAD = NBLK * DBLK                  # 6272
SCALE = 4.0
BIG = 30000.0

F32 = mybir.dt.float32
F16 = mybir.dt.float16
BF16 = mybir.dt.bfloat16

_cache = {}


def _build_program(t_blk, reps):
    """One SPMD program for all 8 cores. t_blk = tiles per dst block (even)."""
    ch = t_blk // 2                      # tiles per DMA chunk
    n_tiles = NBLK * t_blk
    nchunk = NBLK * 2

    nc = bacc.Bacc("TRN2", target_bir_lowering=False, debug=False,
                   enable_asserts=True, num_devices=N_CORES)
    dram = {}

    def din(name, shape, dt):
        dram[name] = nc.dram_tensor(name, shape, dt, kind="ExternalInput").ap()
        return dram[name]

    t_esrcA = din("esrcA", [nchunk, 128, ch * 130], F16)
    t_esrcT = din("esrcT", [nchunk, 128, ch * 128], F16)
    t_dstloc = din("dstloc", [128, n_tiles], F32)
    t_dstT = din("dstT", [128, NDST_PAD], F32)
    t_wqT = din("WqT", [128, 128], F32)
    t_wk = din("Wk", [128, 128], F32)
    t_wvT = din("WvT", [128, 128], F32)
    t_bq = din("bq", [128, 1], F32)
    t_bv = din("bv", [128, 1], F32)
    t_iota = din("iota", [128, 128], F32)
    t_ident = din("ident", [128, 128], F32)
    t_out = nc.dram_tensor("outT", [128, NDST_PAD], F32,
                           kind="ExternalOutput").ap()

    with tile.TileContext(nc) as tc:
        with tc.tile_pool(name="consts", bufs=1) as cpool, \
             tc.tile_pool(name="qp", bufs=1) as qpool, \
             tc.tile_pool(name="aggslab", bufs=1) as apool, \
             tc.tile_pool(name="stream", bufs=3) as spool, \
             tc.tile_pool(name="work", bufs=4) as wpool, \
             tc.tile_pool(name="ps", bufs=4, space="PSUM") as pspool, \
             tc.tile_pool(name="psagg", bufs=2, space="PSUM") as pagpool, \
             tc.tile_pool(name="psmisc", bufs=2, space="PSUM") as pmpool:

            # ---- constants ----
            wqT = cpool.tile([128, 128], F32)
            nc.sync.dma_start(wqT[:], t_wqT[:])
            wk = cpool.tile([128, 128], F32)
            nc.sync.dma_start(wk[:], t_wk[:])
            wvT = cpool.tile([128, 128], F32)
            nc.sync.dma_start(wvT[:], t_wvT[:])
            bq = cpool.tile([128, 1], F32)
            nc.sync.dma_start(bq[:], t_bq[:])
            bv = cpool.tile([128, 1], F32)
            nc.sync.dma_start(bv[:], t_bv[:])
            iota = cpool.tile([128, 128], F32)
            nc.sync.dma_start(iota[:], t_iota[:])
            ident = cpool.tile([128, 128], F32)
            nc.sync.dma_start(ident[:], t_ident[:])
            dstT = cpool.tile([128, NDST_PAD], F32)
            nc.sync.dma_start(dstT[:], t_dstT[:])
            dstloc = cpool.tile([128, n_tiles], F32)
            nc.sync.dma_start(dstloc[:], t_dstloc[:])

            qpT = qpool.tile([128, NDST_PAD], F16)     # Q'^T, f16
            aggT = apool.tile([128, NDST_PAD], F32)    # agg^T slab

            def body(_iv=None):
                # ---- P2+P3: Q^T = Wq @ dstT + bq ; Q'^T = Wk^T @ Q^T ----
                for b in range(NBLK):
                    qt_ps = pmpool.tile([128, 128], F32, tag="mm")
                    nc.tensor.matmul(qt_ps[:], lhsT=wqT[:],
                                     rhs=dstT[:, b * 128:(b + 1) * 128],
                                     start=True, stop=True)
                    qt_sb = wpool.tile([128, 128], F32, tag="qtsb")
                    nc.scalar.activation(qt_sb[:], qt_ps[:],
                                         mybir.ActivationFunctionType.Identity,
                                         bias=bq[:, :1])
                    qp_ps = pmpool.tile([128, 128], F32, tag="mm")
                    nc.tensor.matmul(qp_ps[:], lhsT=wk[:], rhs=qt_sb[:],
                                     start=True, stop=True)
                    nc.scalar.activation(qpT[:, b * 128:(b + 1) * 128], qp_ps[:],
                                         mybir.ActivationFunctionType.Copy)

                # ---- P4: edge phase ----
                for b in range(NBLK):
                    aggP = pagpool.tile([128, 129], F32, tag="agg")
                    chA0 = chA1 = chT0 = chT1 = None
                    for t in range(t_blk):
                        if t == 0:
                            chA0 = spool.tile([128, ch * 130], F16, tag="cA0")
                            nc.sync.dma_start(chA0[:], t_esrcA[2 * b])
                            chT0 = spool.tile([128, ch * 128], F16, tag="cT0")
                            nc.sync.dma_start(chT0[:], t_esrcT[2 * b])
                            chA1 = spool.tile([128, ch * 130], F16, tag="cA1")
                            nc.sync.dma_start(chA1[:], t_esrcA[2 * b + 1])
                            chT1 = spool.tile([128, ch * 128], F16, tag="cT1")
                            nc.sync.dma_start(chT1[:], t_esrcT[2 * b + 1])
                        g = b * t_blk + t
                        cA, cT = (chA0, chT0) if t < ch else (chA1, chT1)
                        tc_ = t if t < ch else t - ch
                        # scores: all-pairs [e, d] for this tile
                        ap_ps = pspool.tile([128, 128], F32, tag="ap")
                        nc.tensor.matmul(
                            ap_ps[:],
                            lhsT=cT[:, tc_ * 128:(tc_ + 1) * 128],
                            rhs=qpT[:, b * 128:(b + 1) * 128],
                            start=True, stop=True)
                        # mask: 0 where iota==dstloc[e], -BIG elsewhere
                        eqm = wpool.tile([128, 128], F32, tag="eqm")
                        nc.vector.tensor_scalar(
                            out=eqm[:], in0=iota[:],
                            scalar1=dstloc[:, g:g + 1], scalar2=-BIG,
                            op0=mybir.AluOpType.not_equal,
                            op1=mybir.AluOpType.mult)
                        sm = wpool.tile([128, 128], F32, tag="sm")
                        nc.vector.tensor_tensor(
                            out=sm[:], in0=ap_ps[:], in1=eqm[:],
                            op=mybir.AluOpType.add)
                        # E = exp(sm / SCALE)  (bf16: range needs ~e^16)
                        E = wpool.tile([128, 128], BF16, tag="E")
                        nc.scalar.activation(E[:], sm[:],
                                             mybir.ActivationFunctionType.Exp,
                                             scale=1.0 / SCALE)
                        # agg[d, 0:128] += E^T @ src ; agg[d,128] += E^T @ 1
                        nc.tensor.matmul(
                            aggP[:],
                            lhsT=E[:],
                            rhs=cA[:, tc_ * 130:tc_ * 130 + 129],
                            start=(t == 0), stop=(t == t_blk - 1))
                    # ---- block end: divide by denom, transpose ----
                    dn = wpool.tile([128, 1], F32, tag="dn")
                    nc.vector.tensor_scalar(
                        out=dn[:], in0=aggP[:, 128:129], scalar1=1e-30,
                        scalar2=None, op0=mybir.AluOpType.max)
                    rc = wpool.tile([128, 1], F32, tag="rc")
                    nc.vector.reciprocal(rc[:], dn[:])
                    aggN = wpool.tile([128, 128], F32, tag="aggN")
                    nc.vector.tensor_scalar(
                        out=aggN[:], in0=aggP[:, :128], scalar1=rc[:, :1],
                        scalar2=None, op0=mybir.AluOpType.mult)
                    tr_ps = pmpool.tile([128, 128], F32, tag="mm")
                    nc.tensor.transpose(tr_ps[:], aggN[:], ident[:])
                    nc.scalar.activation(aggT[:, b * 128:(b + 1) * 128], tr_ps[:],
                                         mybir.ActivationFunctionType.Copy)

                # ---- P5: out^T = Wv @ agg^T + bv ----
                for b in range(NBLK):
                    o_ps = pmpool.tile([128, 128], F32, tag="mm")
                    nc.tensor.matmul(o_ps[:], lhsT=wvT[:],
                                     rhs=aggT[:, b * 128:(b + 1) * 128],
                                     start=True, stop=True)
                    o_sb = wpool.tile([128, 128], F32, tag="osb")
                    nc.scalar.activation(o_sb[:], o_ps[:],
                                         mybir.ActivationFunctionType.Identity,
                                         bias=bv[:, :1])
                    nc.sync.dma_start(t_out[:, b * 128:(b + 1) * 128], o_sb[:])

            if reps == 1:
                body()
            else:
                with tc.For_i(0, reps, 1):
                    body()

    nc.compile()
    return nc


def _prep(src_feat, dst_feat, src_idx, dst_idx, Wq, bq, Wk, bk, Wv, bv):
    """Host-side layout: sort edges by dst, shard by dst range, build tiles."""
    src_feat = np.asarray(src_feat, np.float32)
    dst_feat = np.asarray(dst_feat, np.float32)
    src_idx = np.asarray(src_idx).astype(np.int64)
    dst_idx = np.asarray(dst_idx).astype(np.int64)

    order = np.argsort(dst_idx, kind="stable")
    d_sorted = dst_idx[order]
    s_sorted = src_idx[order]

    core_lo = np.searchsorted(d_sorted, np.arange(N_CORES) * NDST_CORE)
    core_hi = np.searchsorted(d_sorted, (np.arange(N_CORES) + 1) * NDST_CORE)

    # per (core, block) edge counts -> global uniform t_blk
    blk_of_edge = (d_sorted % NDST_CORE) // DBLK  # valid within a core's range
    t_blk = 0
    counts = []
    for c in range(N_CORES):
        cnt = np.bincount(blk_of_edge[core_lo[c]:core_hi[c]], minlength=NBLK)
        counts.append(cnt)
        t_blk = max(t_blk, int(np.ceil(cnt.max() / 128)))
    t_blk = t_blk + (t_blk % 2)  # even
    ch = t_blk // 2
    n_tiles = NBLK * t_blk
    n_slots = n_tiles * 128

    in_maps = []
    for c in range(N_CORES):
        lo, hi = core_lo[c], core_hi[c]
        s_c = s_sorted[lo:hi]
        dloc_c = (d_sorted[lo:hi] % NDST_CORE) % DBLK
        blk_c = blk_of_edge[lo:hi]
        cnt = counts[c]
        # slot index for each edge: block base + position within block
        off_in_blk = np.arange(hi - lo) - np.repeat(
            np.concatenate([[0], np.cumsum(cnt)[:-1]]), cnt)
        slot = blk_c * (t_blk * 128) + off_in_blk

        srcslot = np.zeros(n_slots, np.int64)
        dlocslot = np.full(n_slots, -1.0, np.float32)
        srcslot[slot] = s_c
        dlocslot[slot] = dloc_c.astype(np.float32)

        esrc = src_feat[srcslot].astype(np.float16)          # [n_slots, 128]
        esrc = esrc.reshape(n_tiles, 128, 128)
        # esrcA: [nblk*2, 128, ch*130] with ones column
        eA = np.zeros((n_tiles, 128, 130), np.float16)
        eA[:, :, :128] = esrc
        eA[:, :, 128] = 1.0
        eA = eA.reshape(NBLK * 2, ch, 128, 130).transpose(0, 2, 1, 3)
        eA = np.ascontiguousarray(eA).reshape(NBLK * 2, 128, ch * 130)
        # esrcT: per-tile transpose [fi, e]
        eT = esrc.transpose(0, 2, 1).reshape(NBLK * 2, ch, 128, 128)
        eT = np.ascontiguousarray(eT.transpose(0, 2, 1, 3)).reshape(
            NBLK * 2, 128, ch * 128)

        dstloc = np.ascontiguousarray(
            dlocslot.reshape(n_tiles, 128).T)                # [128, n_tiles]

        dT = np.zeros((128, NDST_PAD), np.float32)
        dT[:, :NDST_CORE] = dst_feat[c * NDST_CORE:(c + 1) * NDST_CORE].T

        in_maps.append({
            "esrcA": eA, "esrcT": eT, "dstloc": dstloc, "dstT": dT,
            "WqT": np.ascontiguousarray(np.asarray(Wq, np.float32).T),
            "Wk": np.ascontiguousarray(np.asarray(Wk, np.float32)),
            "WvT": np.ascontiguousarray(np.asarray(Wv, np.float32).T),
            "bq": np.asarray(bq, np.float32).reshape(128, 1),
            "bv": np.asarray(bv, np.float32).reshape(128, 1),
            "iota": np.tile(np.arange(128, dtype=np.float32), (128, 1)),
            "ident": np.eye(128, dtype=np.float32),
        })
    return in_maps, t_blk, dst_idx


def _run(nc, in_maps):
    from concourse.bass_utils import run_bass_kernel_spmd
    res = run_bass_kernel_spmd(nc, in_maps, list(range(N_CORES)))
    return res.results


def kernel(src_feat, dst_feat, src_idx, dst_idx, Wq, bq, Wk, bk, Wv, bv):
    in_maps, t_blk, dst_idx_np = _prep(src_feat, dst_feat, src_idx, dst_idx,
                                       Wq, bq, Wk, bk, Wv, bv)
    key = (t_blk, 1)
    if key not in _cache:
        _cache[key] = _build_program(t_blk, 1)
    nc = _cache[key]
    results = _run(nc, in_maps)

    out = np.empty((N_NODES, D), np.float32)
    for c in range(N_CORES):
        out[c * NDST_CORE:(c + 1) * NDST_CORE] = \
            results[c]["outT"][:, :NDST_CORE].T
    # degree-0 dst rows: reference yields 0, device yields bv — fix up
    deg = np.bincount(dst_idx_np, minlength=N_NODES)
    if (deg == 0).any():
        out[deg == 0] = 0.0
    return out


# revision 7
# speedup vs baseline: 1.3154x; 1.3154x over previous
"""CrossScaleAttention (GNN segment-softmax attention) on 8 TRN2 NeuronCores.

Math refactor (all FLOPs on device, host only re-lays-out raw inputs):
  score[e] = Q[dst_e] . K[src_e],  Q = dst @ Wq^T + bq,  K = src @ Wk^T + bk
           = Q'[dst_e] . src_feat[src_e] + Q[dst_e].bk   (Q' = Q @ Wk)
  The per-dst constant Q[d].bk cancels in the segment softmax, so only
  Q' (a small per-dst table, computed on device) and raw src_feat rows are
  needed per edge.  Likewise the V projection commutes with the attention-
  weighted sum: out[d] = (sum_e attn_e * src_feat[src_e]) @ Wv^T + bv.

Sharding: dst nodes range-partitioned across 8 cores; edges sorted by dst on
host and laid out edge-major (128-edge tiles x 128-dst blocks, padded to a
uniform tile count per block).  The host ships src_feat rows in both [e, fi]
and [fi, e] tile orientations so the device streams them contiguously at full
DMA bandwidth (no indirect DMA).  Per tile the device computes all-pairs
scores (one matmul), masks+exps them (DVE+ACT), and accumulates the weighted
segment sums (one matmul into PSUM).
"""
import sys
sys.path.insert(0, "/opt/trn_rl_repo")

import numpy as np

import concourse.bass as bass
import concourse.bacc as bacc
import concourse.tile as tile
import concourse.mybir as mybir

N_NODES = 50000
D = 128
N_CORES = 8
NDST_CORE = N_NODES // N_CORES          # 6250
DBLK = 128
NBLK = (NDST_CORE + DBLK - 1) // DBLK   # 49
NDST_PAD = NBLK * DBLK                  # 6272
SCALE = 4.0
BIG = 30000.0

F32 = mybir.dt.float32
F8 = mybir.dt.float8e5
F16 = mybir.dt.float16
BF16 = mybir.dt.bfloat16

_cache = {}


def _build_program(t_blk, reps, ablate=frozenset()):
    """One SPMD program for all 8 cores. t_blk = tiles per dst block (even)."""
    ch = t_blk // 2                      # tiles per DMA chunk
    n_tiles = NBLK * t_blk
    nchunk = NBLK * 2

    nc = bacc.Bacc("TRN2", target_bir_lowering=False, debug=False,
                   enable_asserts=True, num_devices=N_CORES)
    dram = {}

    def din(name, shape, dt):
        dram[name] = nc.dram_tensor(name, shape, dt, kind="ExternalInput").ap()
        return dram[name]

    t_esrcA = din("esrcA", [nchunk, 128, ch * 130], F16)
    t_esrcT = din("esrcT", [nchunk, 128, ch * 128], F16)
    t_maskA = din("maskA", [nchunk, 128, ch * 128], F8)
    t_dstloc = din("dstloc", [128, n_tiles], F32)
    t_dstT = din("dstT", [128, NDST_PAD], F32)
    t_wqT = din("WqT", [128, 128], F32)
    t_wk = din("Wk", [128, 128], F32)
    t_wvT = din("WvT", [128, 128], F32)
    t_bq = din("bq", [128, 1], F32)
    t_bv = din("bv", [128, 1], F32)
    t_iota = din("iota", [128, 128], F32)
    t_ident = din("ident", [128, 128], F32)
    t_out = nc.dram_tensor("outT", [128, NDST_PAD], F32,
                           kind="ExternalOutput").ap()

    with tile.TileContext(nc) as tc:
        with tc.tile_pool(name="consts", bufs=1) as cpool, \
             tc.tile_pool(name="qp", bufs=1) as qpool, \
             tc.tile_pool(name="aggslab", bufs=1) as apool, \
             tc.tile_pool(name="stream", bufs=3) as spool, \
             tc.tile_pool(name="work", bufs=6) as wpool, \
             tc.tile_pool(name="ps", bufs=4, space="PSUM") as pspool, \
             tc.tile_pool(name="psagg", bufs=2, space="PSUM") as pagpool, \
             tc.tile_pool(name="psmisc", bufs=2, space="PSUM") as pmpool:

            # ---- constants ----
            wqT = cpool.tile([128, 128], F32)
            nc.sync.dma_start(wqT[:], t_wqT[:])
            wk = cpool.tile([128, 128], F32)
            nc.sync.dma_start(wk[:], t_wk[:])
            wvT = cpool.tile([128, 128], F32)
            nc.sync.dma_start(wvT[:], t_wvT[:])
            bq = cpool.tile([128, 1], F32)
            nc.sync.dma_start(bq[:], t_bq[:])
            bv = cpool.tile([128, 1], F32)
            nc.sync.dma_start(bv[:], t_bv[:])
            iota = cpool.tile([128, 128], F32)
            nc.sync.dma_start(iota[:], t_iota[:])
            ident = cpool.tile([128, 128], F32)
            nc.sync.dma_start(ident[:], t_ident[:])
            dstT = cpool.tile([128, NDST_PAD], F32)
            nc.sync.dma_start(dstT[:], t_dstT[:])
            dstloc = cpool.tile([128, n_tiles], F32)
            nc.sync.dma_start(dstloc[:], t_dstloc[:])

            qpT = qpool.tile([128, NDST_PAD], F16)     # Q'^T, f16
            aggT = apool.tile([128, NDST_PAD], F32)    # agg^T slab

            def body(_iv=None):
                # ---- P2+P3: Q^T = Wq @ dstT + bq ; Q'^T = Wk^T @ Q^T ----
                for b in range(NBLK):
                    qt_ps = pmpool.tile([128, 128], F32, tag="mm")
                    nc.tensor.matmul(qt_ps[:], lhsT=wqT[:],
                                     rhs=dstT[:, b * 128:(b + 1) * 128],
                                     start=True, stop=True)
                    qt_sb = wpool.tile([128, 128], F32, tag="qtsb")
                    nc.scalar.activation(qt_sb[:], qt_ps[:],
                                         mybir.ActivationFunctionType.Identity,
                                         bias=bq[:, :1])
                    qp_ps = pmpool.tile([128, 128], F32, tag="mm")
                    nc.tensor.matmul(qp_ps[:], lhsT=wk[:], rhs=qt_sb[:],
                                     start=True, stop=True)
                    nc.scalar.activation(qpT[:, b * 128:(b + 1) * 128], qp_ps[:],
                                         mybir.ActivationFunctionType.Copy)

                # ---- P4: edge phase ----
                for b in range(NBLK):
                    aggP = pagpool.tile([128, 129], F32, tag="agg")
                    chA0 = chA1 = chT0 = chT1 = None
                    for t in range(t_blk):
                        if t == 0:
                            chA0 = spool.tile([128, ch * 130], F16, tag="cA0")
                            chT0 = spool.tile([128, ch * 128], F16, tag="cT0")
                            chA1 = spool.tile([128, ch * 130], F16, tag="cA1")
                            chT1 = spool.tile([128, ch * 128], F16, tag="cT1")
                            chM0 = spool.tile([128, ch * 128], F8, tag="cM0")
                            chM1 = spool.tile([128, ch * 128], F8, tag="cM1")
                            nc.sync.dma_start(chA0[:], t_esrcA[2 * b])
                            nc.sync.dma_start(chT0[:], t_esrcT[2 * b])
                            nc.sync.dma_start(chM0[:], t_maskA[2 * b])
                            nc.sync.dma_start(chA1[:], t_esrcA[2 * b + 1])
                            nc.sync.dma_start(chT1[:], t_esrcT[2 * b + 1])
                            nc.sync.dma_start(chM1[:], t_maskA[2 * b + 1])
                        g = b * t_blk + t
                        cA, cT, cM = (chA0, chT0, chM0) if t < ch else (chA1, chT1, chM1)
                        tc_ = t if t < ch else t - ch
                        # scores: all-pairs [e, d] for this tile
                        ap_ps = pspool.tile([128, 128], F32, tag="ap")
                        if "score" not in ablate:
                            nc.tensor.matmul(
                                ap_ps[:],
                                lhsT=cT[:, tc_ * 128:(tc_ + 1) * 128],
                                rhs=qpT[:, b * 128:(b + 1) * 128],
                                start=True, stop=True)
                        else:
                            nc.tensor.matmul(
                                ap_ps[:], lhsT=wk[:], rhs=iota[:],
                                start=True, stop=True)
                        # mask: 0 where dstloc[e]==d, -BIG elsewhere (host fp8)
                        if "dve" not in ablate:
                            sm = wpool.tile([128, 128], F32, tag="sm")
                            nc.vector.tensor_tensor(
                                out=sm[:], in0=ap_ps[:],
                                in1=cM[:, tc_ * 128:(tc_ + 1) * 128],
                                op=mybir.AluOpType.add)
                        else:
                            sm = ap_ps
                        # E = exp(sm / SCALE)  (bf16: range needs ~e^16)
                        E = wpool.tile([128, 128], BF16, tag="E")
                        if "act" not in ablate:
                            nc.scalar.activation(E[:], sm[:],
                                                 mybir.ActivationFunctionType.Exp,
                                                 scale=1.0 / SCALE)
                        elif t == 0:
                            nc.scalar.activation(E[:], iota[:],
                                                 mybir.ActivationFunctionType.Exp,
                                                 scale=1.0 / SCALE)
                        # agg[d, 0:128] += E^T @ src ; agg[d,128] += E^T @ 1
                        if "agg" not in ablate:
                            nc.tensor.matmul(
                                aggP[:],
                                lhsT=E[:],
                                rhs=cA[:, tc_ * 130:tc_ * 130 + 129],
                                start=(t == 0), stop=(t == t_blk - 1))
                        elif t == 0:
                            nc.tensor.matmul(
                                aggP[:], lhsT=E[:],
                                rhs=cA[:, 0:129],
                                start=True, stop=True)
                    # ---- block end: divide by denom, transpose ----
                    dn = wpool.tile([128, 1], F32, tag="dn")
                    nc.vector.tensor_scalar(
                        out=dn[:], in0=aggP[:, 128:129], scalar1=1e-30,
                        scalar2=None, op0=mybir.AluOpType.max)
                    rc = wpool.tile([128, 1], F32, tag="rc")
                    nc.vector.reciprocal(rc[:], dn[:])
                    aggN = wpool.tile([128, 128], F32, tag="aggN")
                    nc.vector.tensor_scalar(
                        out=aggN[:], in0=aggP[:, :128], scalar1=rc[:, :1],
                        scalar2=None, op0=mybir.AluOpType.mult)
                    tr_ps = pmpool.tile([128, 128], F32, tag="mm")
                    nc.tensor.transpose(tr_ps[:], aggN[:], ident[:])
                    nc.scalar.activation(aggT[:, b * 128:(b + 1) * 128], tr_ps[:],
                                         mybir.ActivationFunctionType.Copy)

                # ---- P5: out^T = Wv @ agg^T + bv ----
                for b in range(NBLK):
                    o_ps = pmpool.tile([128, 128], F32, tag="mm")
                    nc.tensor.matmul(o_ps[:], lhsT=wvT[:],
                                     rhs=aggT[:, b * 128:(b + 1) * 128],
                                     start=True, stop=True)
                    o_sb = wpool.tile([128, 128], F32, tag="osb")
                    nc.scalar.activation(o_sb[:], o_ps[:],
                                         mybir.ActivationFunctionType.Identity,
                                         bias=bv[:, :1])
                    nc.sync.dma_start(t_out[:, b * 128:(b + 1) * 128], o_sb[:])

            if reps == 1:
                body()
            else:
                with tc.For_i(0, reps, 1):
                    body()

    nc.compile()
    return nc


def _prep(src_feat, dst_feat, src_idx, dst_idx, Wq, bq, Wk, bk, Wv, bv):
    """Host-side layout: sort edges by dst, shard by dst range, build tiles."""
    src_feat = np.asarray(src_feat, np.float32)
    dst_feat = np.asarray(dst_feat, np.float32)
    src_idx = np.asarray(src_idx).astype(np.int64)
    dst_idx = np.asarray(dst_idx).astype(np.int64)

    order = np.argsort(dst_idx, kind="stable")
    d_sorted = dst_idx[order]
    s_sorted = src_idx[order]

    core_lo = np.searchsorted(d_sorted, np.arange(N_CORES) * NDST_CORE)
    core_hi = np.searchsorted(d_sorted, (np.arange(N_CORES) + 1) * NDST_CORE)

    # per (core, block) edge counts -> global uniform t_blk
    blk_of_edge = (d_sorted % NDST_CORE) // DBLK  # valid within a core's range
    t_blk = 0
    counts = []
    for c in range(N_CORES):
        cnt = np.bincount(blk_of_edge[core_lo[c]:core_hi[c]], minlength=NBLK)
        counts.append(cnt)
        t_blk = max(t_blk, int(np.ceil(cnt.max() / 128)))
    t_blk = t_blk + (t_blk % 2)  # even
    ch = t_blk // 2
    n_tiles = NBLK * t_blk
    n_slots = n_tiles * 128

    in_maps = []
    for c in range(N_CORES):
        lo, hi = core_lo[c], core_hi[c]
        s_c = s_sorted[lo:hi]
        dloc_c = (d_sorted[lo:hi] % NDST_CORE) % DBLK
        blk_c = blk_of_edge[lo:hi]
        cnt = counts[c]
        # slot index for each edge: block base + position within block
        off_in_blk = np.arange(hi - lo) - np.repeat(
            np.concatenate([[0], np.cumsum(cnt)[:-1]]), cnt)
        slot = blk_c * (t_blk * 128) + off_in_blk

        srcslot = np.zeros(n_slots, np.int64)
        dlocslot = np.full(n_slots, -1.0, np.float32)
        srcslot[slot] = s_c
        dlocslot[slot] = dloc_c.astype(np.float32)

        esrc = src_feat[srcslot].astype(np.float16)          # [n_slots, 128]
        esrc = esrc.reshape(n_tiles, 128, 128)
        import ml_dtypes
        mA = np.where(dlocslot[:, None] == np.arange(128, dtype=np.float32)[None, :],
                      np.float32(0.0), np.float32(-BIG)).astype(ml_dtypes.float8_e5m2)
        mA = mA.reshape(NBLK * 2, ch, 128, 128).transpose(0, 2, 1, 3)
        mA = np.ascontiguousarray(mA).reshape(NBLK * 2, 128, ch * 128)
        # esrcA: [nblk*2, 128, ch*130] with ones column
        eA = np.zeros((n_tiles, 128, 130), np.float16)
        eA[:, :, :128] = esrc
        eA[:, :, 128] = 1.0
        eA = eA.reshape(NBLK * 2, ch, 128, 130).transpose(0, 2, 1, 3)
        eA = np.ascontiguousarray(eA).reshape(NBLK * 2, 128, ch * 130)
        # esrcT: per-tile transpose [fi, e]
        eT = esrc.transpose(0, 2, 1).reshape(NBLK * 2, ch, 128, 128)
        eT = np.ascontiguousarray(eT.transpose(0, 2, 1, 3)).reshape(
            NBLK * 2, 128, ch * 128)

        dstloc = np.ascontiguousarray(
            dlocslot.reshape(n_tiles, 128).T)                # [128, n_tiles]

        dT = np.zeros((128, NDST_PAD), np.float32)
        dT[:, :NDST_CORE] = dst_feat[c * NDST_CORE:(c + 1) * NDST_CORE].T

        in_maps.append({
            "esrcA": eA, "esrcT": eT, "maskA": mA, "dstloc": dstloc, "dstT": dT,
            "WqT": np.ascontiguousarray(np.asarray(Wq, np.float32).T),
            "Wk": np.ascontiguousarray(np.asarray(Wk, np.float32)),
            "WvT": np.ascontiguousarray(np.asarray(Wv, np.float32).T),
            "bq": np.asarray(bq, np.float32).reshape(128, 1),
            "bv": np.asarray(bv, np.float32).reshape(128, 1),
            "iota": np.tile(np.arange(128, dtype=np.float32), (128, 1)),
            "ident": np.eye(128, dtype=np.float32),
        })
    return in_maps, t_blk, dst_idx


def _run(nc, in_maps):
    from concourse.bass_utils import run_bass_kernel_spmd
    res = run_bass_kernel_spmd(nc, in_maps, list(range(N_CORES)))
    return res.results


def kernel(src_feat, dst_feat, src_idx, dst_idx, Wq, bq, Wk, bk, Wv, bv):
    in_maps, t_blk, dst_idx_np = _prep(src_feat, dst_feat, src_idx, dst_idx,
                                       Wq, bq, Wk, bk, Wv, bv)
    key = (t_blk, 1)
    if key not in _cache:
        _cache[key] = _build_program(t_blk, 1)
    nc = _cache[key]
    results = _run(nc, in_maps)

    out = np.empty((N_NODES, D), np.float32)
    for c in range(N_CORES):
        out[c * NDST_CORE:(c + 1) * NDST_CORE] = \
            results[c]["outT"][:, :NDST_CORE].T
    # degree-0 dst rows: reference yields 0, device yields bv — fix up
    deg = np.bincount(dst_idx_np, minlength=N_NODES)
    if (deg == 0).any():
        out[deg == 0] = 0.0
    return out


# revision 8
# speedup vs baseline: 1.6224x; 1.2333x over previous
"""CrossScaleAttention (GNN segment-softmax attention) on 8 TRN2 NeuronCores.

Math refactor (all FLOPs on device, host only re-lays-out raw inputs):
  score[e] = Q[dst_e] . K[src_e],  Q = dst @ Wq^T + bq,  K = src @ Wk^T + bk
           = Q'[dst_e] . src_feat[src_e] + Q[dst_e].bk   (Q' = Q @ Wk)
  The per-dst constant Q[d].bk cancels in the segment softmax, so only
  Q' (a small per-dst table, computed on device) and raw src_feat rows are
  needed per edge.  Likewise the V projection commutes with the attention-
  weighted sum: out[d] = (sum_e attn_e * src_feat[src_e]) @ Wv^T + bv.

Sharding: dst nodes range-partitioned across 8 cores; edges sorted by dst on
host and laid out edge-major (128-edge tiles x 128-dst blocks, padded to a
uniform tile count per block).  The host ships src_feat rows in both [e, fi]
and [fi, e] tile orientations so the device streams them contiguously at full
DMA bandwidth (no indirect DMA).  Per tile the device computes all-pairs
scores (one matmul), masks+exps them (DVE+ACT), and accumulates the weighted
segment sums (one matmul into PSUM).
"""
import sys
sys.path.insert(0, "/opt/trn_rl_repo")

import numpy as np

import concourse.bass as bass
import concourse.bacc as bacc
import concourse.tile as tile
import concourse.mybir as mybir

N_NODES = 50000
D = 128
N_CORES = 8
NDST_CORE = N_NODES // N_CORES          # 6250
DBLK = 128
NBLK = (NDST_CORE + DBLK - 1) // DBLK   # 49
NDST_PAD = NBLK * DBLK                  # 6272
SCALE = 4.0
BIG = 30000.0

F32 = mybir.dt.float32
F8 = mybir.dt.float8e5
F16 = mybir.dt.float16
BF16 = mybir.dt.bfloat16

_cache = {}


def _build_program(t_blk, reps, ablate=frozenset()):
    """One SPMD program for all 8 cores. t_blk = tiles per dst block (even)."""
    ch = t_blk // 2                      # tiles per DMA chunk
    n_tiles = NBLK * t_blk
    nchunk = NBLK * 2

    nc = bacc.Bacc("TRN2", target_bir_lowering=False, debug=False,
                   enable_asserts=True, num_devices=N_CORES)
    dram = {}

    def din(name, shape, dt):
        dram[name] = nc.dram_tensor(name, shape, dt, kind="ExternalInput").ap()
        return dram[name]

    t_esrcA = din("esrcA", [nchunk, 128, ch * 130], F16)
    t_esrcT = din("esrcT", [nchunk, 128, ch * 128], F16)
    t_maskA = din("maskA", [nchunk, 128, ch * 128], F8)
    t_dstloc = din("dstloc", [128, n_tiles], F32)
    t_dstT = din("dstT", [128, NDST_PAD], F32)
    t_wqT = din("WqT", [128, 128], F32)
    t_wk = din("Wk", [128, 128], F32)
    t_wvT = din("WvT", [128, 128], F32)
    t_bq = din("bq", [128, 1], F32)
    t_bv = din("bv", [128, 1], F32)
    t_iota = din("iota", [128, 128], F32)
    t_ident = din("ident", [128, 128], F32)
    t_out = nc.dram_tensor("outT", [128, NDST_PAD], F32,
                           kind="ExternalOutput").ap()

    with tile.TileContext(nc) as tc:
        with tc.tile_pool(name="consts", bufs=1) as cpool, \
             tc.tile_pool(name="qp", bufs=1) as qpool, \
             tc.tile_pool(name="aggslab", bufs=1) as apool, \
             tc.tile_pool(name="stream", bufs=3) as spool, \
             tc.tile_pool(name="work", bufs=6) as wpool, \
             tc.tile_pool(name="ps", bufs=4, space="PSUM") as pspool, \
             tc.tile_pool(name="psagg", bufs=2, space="PSUM") as pagpool, \
             tc.tile_pool(name="psmisc", bufs=2, space="PSUM") as pmpool:

            # ---- constants ----
            wqT = cpool.tile([128, 128], F32)
            nc.sync.dma_start(wqT[:], t_wqT[:])
            wk = cpool.tile([128, 128], F32)
            nc.sync.dma_start(wk[:], t_wk[:])
            wvT = cpool.tile([128, 128], F32)
            nc.sync.dma_start(wvT[:], t_wvT[:])
            bq = cpool.tile([128, 1], F32)
            nc.sync.dma_start(bq[:], t_bq[:])
            bv = cpool.tile([128, 1], F32)
            nc.sync.dma_start(bv[:], t_bv[:])
            iota = cpool.tile([128, 128], F32)
            nc.sync.dma_start(iota[:], t_iota[:])
            ident = cpool.tile([128, 128], F32)
            nc.sync.dma_start(ident[:], t_ident[:])
            dstT = cpool.tile([128, NDST_PAD], F32)
            nc.sync.dma_start(dstT[:], t_dstT[:])
            dstloc = cpool.tile([128, n_tiles], F32)
            nc.sync.dma_start(dstloc[:], t_dstloc[:])

            qpT = qpool.tile([128, NDST_PAD], F16)     # Q'^T, f16
            aggT = apool.tile([128, NDST_PAD], F32)    # agg^T slab

            def body(_iv=None):
                # ---- P2+P3: Q^T = Wq @ dstT + bq ; Q'^T = Wk^T @ Q^T ----
                for b in range(NBLK):
                    qt_ps = pmpool.tile([128, 128], F32, tag="mm")
                    nc.tensor.matmul(qt_ps[:], lhsT=wqT[:],
                                     rhs=dstT[:, b * 128:(b + 1) * 128],
                                     start=True, stop=True)
                    qt_sb = wpool.tile([128, 128], F32, tag="qtsb")
                    nc.scalar.activation(qt_sb[:], qt_ps[:],
                                         mybir.ActivationFunctionType.Identity,
                                         bias=bq[:, :1])
                    qp_ps = pmpool.tile([128, 128], F32, tag="mm")
                    nc.tensor.matmul(qp_ps[:], lhsT=wk[:], rhs=qt_sb[:],
                                     start=True, stop=True)
                    nc.scalar.activation(qpT[:, b * 128:(b + 1) * 128], qp_ps[:],
                                         mybir.ActivationFunctionType.Copy)

                # ---- P4: edge phase ----
                for b in range(NBLK):
                    aggP = pagpool.tile([128, 129], F32, tag="agg")
                    chA0 = chA1 = chT0 = chT1 = None
                    for t in range(0, t_blk, 2):
                        if t == 0:
                            chA0 = spool.tile([128, ch * 130], F16, tag="cA0")
                            chT0 = spool.tile([128, ch * 128], F16, tag="cT0")
                            chA1 = spool.tile([128, ch * 130], F16, tag="cA1")
                            chT1 = spool.tile([128, ch * 128], F16, tag="cT1")
                            chM0 = spool.tile([128, ch * 128], F8, tag="cM0")
                            chM1 = spool.tile([128, ch * 128], F8, tag="cM1")
                            nc.sync.dma_start(chA0[:], t_esrcA[2 * b])
                            nc.sync.dma_start(chT0[:], t_esrcT[2 * b])
                            nc.sync.dma_start(chM0[:], t_maskA[2 * b])
                            nc.sync.dma_start(chA1[:], t_esrcA[2 * b + 1])
                            nc.sync.dma_start(chT1[:], t_esrcT[2 * b + 1])
                            nc.sync.dma_start(chM1[:], t_maskA[2 * b + 1])
                        # pair of tiles (t, t+1): shared PSUM/DVE/ACT
                        ap_ps = pspool.tile([128, 256], F32, tag="ap")
                        for j in (0, 1):
                            tj = t + j
                            cT = chT0 if tj < ch else chT1
                            tcj = tj if tj < ch else tj - ch
                            nc.tensor.matmul(
                                ap_ps[:, j * 128:(j + 1) * 128],
                                lhsT=cT[:, tcj * 128:(tcj + 1) * 128],
                                rhs=qpT[:, b * 128:(b + 1) * 128],
                                start=True, stop=True)
                        # mask add (host fp8) + exp for both tiles at once
                        sm = wpool.tile([128, 256], F32, tag="sm")
                        cM = chM0 if t < ch else chM1
                        cM2 = chM0 if t + 1 < ch else chM1
                        tc_ = t if t < ch else t - ch
                        tc2 = t + 1 if t + 1 < ch else t + 1 - ch
                        if tc2 == tc_ + 1:
                            nc.vector.tensor_tensor(
                                out=sm[:], in0=ap_ps[:],
                                in1=cM[:, tc_ * 128:(tc_ + 2) * 128],
                                op=mybir.AluOpType.add)
                        else:
                            nc.vector.tensor_tensor(
                                out=sm[:, :128], in0=ap_ps[:, :128],
                                in1=cM[:, tc_ * 128:(tc_ + 1) * 128],
                                op=mybir.AluOpType.add)
                            nc.vector.tensor_tensor(
                                out=sm[:, 128:], in0=ap_ps[:, 128:],
                                in1=cM2[:, tc2 * 128:(tc2 + 1) * 128],
                                op=mybir.AluOpType.add)
                        E = wpool.tile([128, 256], BF16, tag="E")
                        nc.scalar.activation(E[:], sm[:],
                                             mybir.ActivationFunctionType.Exp,
                                             scale=1.0 / SCALE)
                        for j in (0, 1):
                            tj = t + j
                            cA = chA0 if tj < ch else chA1
                            tcj = tj if tj < ch else tj - ch
                            nc.tensor.matmul(
                                aggP[:],
                                lhsT=E[:, j * 128:(j + 1) * 128],
                                rhs=cA[:, tcj * 130:tcj * 130 + 129],
                                start=(tj == 0), stop=(tj == t_blk - 1))
                    # ---- block end: divide by denom, transpose ----
                    dn = wpool.tile([128, 1], F32, tag="dn")
                    nc.vector.tensor_scalar(
                        out=dn[:], in0=aggP[:, 128:129], scalar1=1e-30,
                        scalar2=None, op0=mybir.AluOpType.max)
                    rc = wpool.tile([128, 1], F32, tag="rc")
                    nc.vector.reciprocal(rc[:], dn[:])
                    aggN = wpool.tile([128, 128], F32, tag="aggN")
                    nc.vector.tensor_scalar(
                        out=aggN[:], in0=aggP[:, :128], scalar1=rc[:, :1],
                        scalar2=None, op0=mybir.AluOpType.mult)
                    tr_ps = pmpool.tile([128, 128], F32, tag="mm")
                    nc.tensor.transpose(tr_ps[:], aggN[:], ident[:])
                    nc.scalar.activation(aggT[:, b * 128:(b + 1) * 128], tr_ps[:],
                                         mybir.ActivationFunctionType.Copy)

                # ---- P5: out^T = Wv @ agg^T + bv ----
                for b in range(NBLK):
                    o_ps = pmpool.tile([128, 128], F32, tag="mm")
                    nc.tensor.matmul(o_ps[:], lhsT=wvT[:],
                                     rhs=aggT[:, b * 128:(b + 1) * 128],
                                     start=True, stop=True)
                    o_sb = wpool.tile([128, 128], F32, tag="osb")
                    nc.scalar.activation(o_sb[:], o_ps[:],
                                         mybir.ActivationFunctionType.Identity,
                                         bias=bv[:, :1])
                    nc.sync.dma_start(t_out[:, b * 128:(b + 1) * 128], o_sb[:])

            if reps == 1:
                body()
            else:
                with tc.For_i(0, reps, 1):
                    body()

    nc.compile()
    return nc


def _prep(src_feat, dst_feat, src_idx, dst_idx, Wq, bq, Wk, bk, Wv, bv):
    """Host-side layout: sort edges by dst, shard by dst range, build tiles."""
    src_feat = np.asarray(src_feat, np.float32)
    dst_feat = np.asarray(dst_feat, np.float32)
    src_idx = np.asarray(src_idx).astype(np.int64)
    dst_idx = np.asarray(dst_idx).astype(np.int64)

    order = np.argsort(dst_idx, kind="stable")
    d_sorted = dst_idx[order]
    s_sorted = src_idx[order]

    core_lo = np.searchsorted(d_sorted, np.arange(N_CORES) * NDST_CORE)
    core_hi = np.searchsorted(d_sorted, (np.arange(N_CORES) + 1) * NDST_CORE)

    # per (core, block) edge counts -> global uniform t_blk
    blk_of_edge = (d_sorted % NDST_CORE) // DBLK  # valid within a core's range
    t_blk = 0
    counts = []
    for c in range(N_CORES):
        cnt = np.bincount(blk_of_edge[core_lo[c]:core_hi[c]], minlength=NBLK)
        counts.append(cnt)
        t_blk = max(t_blk, int(np.ceil(cnt.max() / 128)))
    t_blk = t_blk + (t_blk % 2)  # even
    ch = t_blk // 2
    n_tiles = NBLK * t_blk
    n_slots = n_tiles * 128

    in_maps = []
    for c in range(N_CORES):
        lo, hi = core_lo[c], core_hi[c]
        s_c = s_sorted[lo:hi]
        dloc_c = (d_sorted[lo:hi] % NDST_CORE) % DBLK
        blk_c = blk_of_edge[lo:hi]
        cnt = counts[c]
        # slot index for each edge: block base + position within block
        off_in_blk = np.arange(hi - lo) - np.repeat(
            np.concatenate([[0], np.cumsum(cnt)[:-1]]), cnt)
        slot = blk_c * (t_blk * 128) + off_in_blk

        srcslot = np.zeros(n_slots, np.int64)
        dlocslot = np.full(n_slots, -1.0, np.float32)
        srcslot[slot] = s_c
        dlocslot[slot] = dloc_c.astype(np.float32)

        esrc = src_feat[srcslot].astype(np.float16)          # [n_slots, 128]
        esrc = esrc.reshape(n_tiles, 128, 128)
        import ml_dtypes
        mA = np.where(dlocslot[:, None] == np.arange(128, dtype=np.float32)[None, :],
                      np.float32(0.0), np.float32(-BIG)).astype(ml_dtypes.float8_e5m2)
        mA = mA.reshape(NBLK * 2, ch, 128, 128).transpose(0, 2, 1, 3)
        mA = np.ascontiguousarray(mA).reshape(NBLK * 2, 128, ch * 128)
        # esrcA: [nblk*2, 128, ch*130] with ones column
        eA = np.zeros((n_tiles, 128, 130), np.float16)
        eA[:, :, :128] = esrc
        eA[:, :, 128] = 1.0
        eA = eA.reshape(NBLK * 2, ch, 128, 130).transpose(0, 2, 1, 3)
        eA = np.ascontiguousarray(eA).reshape(NBLK * 2, 128, ch * 130)
        # esrcT: per-tile transpose [fi, e]
        eT = esrc.transpose(0, 2, 1).reshape(NBLK * 2, ch, 128, 128)
        eT = np.ascontiguousarray(eT.transpose(0, 2, 1, 3)).reshape(
            NBLK * 2, 128, ch * 128)

        dstloc = np.ascontiguousarray(
            dlocslot.reshape(n_tiles, 128).T)                # [128, n_tiles]

        dT = np.zeros((128, NDST_PAD), np.float32)
        dT[:, :NDST_CORE] = dst_feat[c * NDST_CORE:(c + 1) * NDST_CORE].T

        in_maps.append({
            "esrcA": eA, "esrcT": eT, "maskA": mA, "dstloc": dstloc, "dstT": dT,
            "WqT": np.ascontiguousarray(np.asarray(Wq, np.float32).T),
            "Wk": np.ascontiguousarray(np.asarray(Wk, np.float32)),
            "WvT": np.ascontiguousarray(np.asarray(Wv, np.float32).T),
            "bq": np.asarray(bq, np.float32).reshape(128, 1),
            "bv": np.asarray(bv, np.float32).reshape(128, 1),
            "iota": np.tile(np.arange(128, dtype=np.float32), (128, 1)),
            "ident": np.eye(128, dtype=np.float32),
        })
    return in_maps, t_blk, dst_idx


def _run(nc, in_maps):
    from concourse.bass_utils import run_bass_kernel_spmd
    res = run_bass_kernel_spmd(nc, in_maps, list(range(N_CORES)))
    return res.results


def kernel(src_feat, dst_feat, src_idx, dst_idx, Wq, bq, Wk, bk, Wv, bv):
    in_maps, t_blk, dst_idx_np = _prep(src_feat, dst_feat, src_idx, dst_idx,
                                       Wq, bq, Wk, bk, Wv, bv)
    key = (t_blk, 1)
    if key not in _cache:
        _cache[key] = _build_program(t_blk, 1)
    nc = _cache[key]
    results = _run(nc, in_maps)

    out = np.empty((N_NODES, D), np.float32)
    for c in range(N_CORES):
        out[c * NDST_CORE:(c + 1) * NDST_CORE] = \
            results[c]["outT"][:, :NDST_CORE].T
    # degree-0 dst rows: reference yields 0, device yields bv — fix up
    deg = np.bincount(dst_idx_np, minlength=N_NODES)
    if (deg == 0).any():
        out[deg == 0] = 0.0
    return out
